# revision 6
# baseline (speedup 1.0000x reference)
"""Llama attention (B=1, S=2048, H=32, KVH=8, D=128) on 8 Trainium2 NeuronCores.

Strategy: tensor-parallel over heads. Core c owns q-heads 4c..4c+3 and kv-head c
(GQA repeat_interleave => q-head g uses kv-head g//4). Everything on-chip stays in
feature-major ("transposed") layout so no activation transposes are needed:

  host:  X^T, Wq_c^T, Wk_c^T, Wv_c^T, Wo^T[:,cols_c]  (pre-transposed, bf16)
  chip:  Q^T = (Wq_c^T)^T-matmuls, K^T, V^T -> V via PE transpose
         RoPE applied in [d, s] layout (rotate-half = partition-half swap)
         S^T[k,q] = K^T-tile^T @ Q^T   (causal: skip fully-masked k-tiles)
         P^T = exp(scale*S^T - 10)     (global shift; cancels in normalization)
         attn^T[d,q] += V-tile^T... = lhsT(V[k,d])^T @ P^T[k,q]
         l[q] += ones^T @ P^T  ;  attn^T *= 1/l
         AllGather(attn^T, 2.1MB/rank) -> A^T[4096, 2048]
         O^T[cols_c] = (Wo^T-tiles)^T @ A^T
  host:  concat O^T col-slices, transpose -> [1, 2048, 4096] fp32

Inputs whose mask is not causal (or non-arange positions feeding a non-table RoPE —
impossible here since tables are built from position_ids) fall back to numpy.
"""

import sys

for _p in ("/opt/trn_rl_repo", "/root/.axon_site/_ro/trn_rl_repo"):
    if _p not in sys.path:
        sys.path.insert(0, _p)

import numpy as np
import ml_dtypes

B, S, HID = 1, 2048, 4096
H, KVH, D = 32, 8, 128
THETA = 10000.0
NC = 8                      # cores
HPC = H // NC               # q-heads per core = 4
FC = HPC * D                # features per core = 512
SC = 512                    # seq chunk (matmul N)
NSC = S // SC               # 4
NKT = S // D                # 16 k-tiles of 128
NJ = HID // 128             # 32 contraction tiles
SCALE = 1.0 / np.sqrt(np.float32(D))
EXP_SHIFT = -10.0

_BF16 = ml_dtypes.bfloat16

_compiled = None


def _build_nc():
    import concourse.bacc as bacc
    import concourse.mybir as mybir
    import concourse.tile as tile
    from concourse.masks import make_identity

    f32 = mybir.dt.float32
    bf16 = mybir.dt.bfloat16

    nc = bacc.Bacc("TRN2", target_bir_lowering=False, debug=False, num_devices=NC)

    xT = nc.dram_tensor("xT", [HID, S], bf16, kind="ExternalInput")
    wqT = nc.dram_tensor("wqT", [HID, FC], bf16, kind="ExternalInput")
    wkT = nc.dram_tensor("wkT", [HID, D], bf16, kind="ExternalInput")
    wvT = nc.dram_tensor("wvT", [HID, D], bf16, kind="ExternalInput")
    woT = nc.dram_tensor("woT", [HID, FC], bf16, kind="ExternalInput")
    cosT = nc.dram_tensor("cosT", [D, S], bf16, kind="ExternalInput")
    sinT = nc.dram_tensor("sinT", [D, S], bf16, kind="ExternalInput")
    dmask = nc.dram_tensor("dmask", [D, 4 * SC], bf16, kind="ExternalInput")
    outT = nc.dram_tensor("outT", [FC, S], f32, kind="ExternalOutput")

    ag_in = nc.dram_tensor("ag_in", [FC, S], bf16)
    ag_out = nc.dram_tensor("ag_out", [HID, S], bf16, addr_space="Shared")

    Exp = mybir.ActivationFunctionType.Exp

    with tile.TileContext(nc) as tc:
        with (
            tc.tile_pool(name="const", bufs=1) as constp,
            tc.tile_pool(name="wo", bufs=NJ) as wop,
        ):
            cos_sb = constp.tile([D, S], bf16, tag="cos")
            sin_sb = constp.tile([D, S], bf16, tag="sin")
            dm_sb = constp.tile([D, 4 * SC], bf16, tag="dm")
            ident_sb = constp.tile([128, 128], bf16, tag="id")
            ones_sb = constp.tile([128, 1], bf16, tag="ones")
            bias_sb = constp.tile([128, 1], mybir.dt.float32, tag="bias")
            ones_row = constp.tile([1, 128], mybir.dt.float32, tag="ones_row")
            nc.sync.dma_start(out=cos_sb[:], in_=cosT[:])
            nc.sync.dma_start(out=sin_sb[:], in_=sinT[:])
            nc.sync.dma_start(out=dm_sb[:], in_=dmask[:])
            make_identity(nc, ident_sb[:])
            nc.vector.memset(ones_sb[:], 1.0)
            nc.vector.memset(bias_sb[:], EXP_SHIFT)
            nc.vector.memset(ones_row[:], 1.0)

            with tc.tile_pool(name="qkv", bufs=1) as qkvp:
                q_sb = [qkvp.tile([D, S], bf16, tag=f"q{h}", name=f"q{h}") for h in range(HPC)]
                k_sb = qkvp.tile([D, S], bf16, tag="k")
                v_sb = qkvp.tile([128, S], bf16, tag="v")  # [seq-part, d] per 128-tile

                # ---------------- phase 1: projections + RoPE ----------------
                with (
                    tc.tile_pool(name="w", bufs=NJ) as wp,
                    tc.tile_pool(name="xt", bufs=NJ) as xtp,
                    tc.tile_pool(name="ps1", bufs=2, space="PSUM") as ps1,
                    tc.tile_pool(name="pstr", bufs=2, space="PSUM") as pstr,
                    tc.tile_pool(name="rope", bufs=3) as ropep,
                ):
                    wq_sb = [wp.tile([128, FC], bf16, tag="wq", name=f"wq{_}") for _ in range(NJ)]
                    wk_sb = [wp.tile([128, D], bf16, tag="wk", name=f"wk{_}") for _ in range(NJ)]
                    wv_sb = [wp.tile([128, D], bf16, tag="wv", name=f"wv{_}") for _ in range(NJ)]
                    for j in range(NJ):
                        r = slice(j * 128, (j + 1) * 128)
                        nc.sync.dma_start(out=wq_sb[j][:], in_=wqT[r, :])
                        nc.sync.dma_start(out=wk_sb[j][:], in_=wkT[r, :])
                        nc.sync.dma_start(out=wv_sb[j][:], in_=wvT[r, :])

                    def rope(ps, dst_ap, cols):
                        """ps: [128, SC] psum fp32 (feature-major); writes dst_ap (bf16)."""
                        base = ropep.tile([D, SC], bf16, tag="r0", name="r0")
                        nc.scalar.copy(base[:], ps[:])
                        shf = ropep.tile([D, SC], bf16, tag="r1", name="r1")
                        nc.sync.dma_start(out=shf[0:64, :], in_=base[64:128, :])
                        nc.sync.dma_start(out=shf[64:128, :], in_=base[0:64, :])
                        t1 = ropep.tile([D, SC], bf16, tag="r2", name="r2")
                        nc.vector.tensor_mul(t1[:], base[:], cos_sb[:, cols])
                        t2 = ropep.tile([D, SC], bf16, tag="r3", name="r3")
                        nc.vector.tensor_mul(t2[:], shf[:], sin_sb[:, cols])
                        nc.vector.tensor_add(dst_ap, t1[:], t2[:])

                    for hs in range(2):  # stream X^T in two seq halves
                        half = slice(hs * 1024, (hs + 1) * 1024)
                        xt_sb = [xtp.tile([128, 1024], bf16, tag="xt", name=f"xt{_}") for _ in range(NJ)]
                        for j in range(NJ):
                            nc.sync.dma_start(
                                out=xt_sb[j][:], in_=xT[j * 128 : (j + 1) * 128, half]
                            )
                        for scl in range(2):
                            sc = 2 * hs + scl
                            cols = slice(sc * SC, (sc + 1) * SC)
                            lcol = slice(scl * SC, (scl + 1) * SC)
                            # Q^T per head
                            for h in range(HPC):
                                ps = ps1.tile([128, SC], f32, tag="ps", name="ps")
                                for j in range(NJ):
                                    nc.tensor.matmul(
                                        ps[:],
                                        wq_sb[j][:, h * 128 : (h + 1) * 128],
                                        xt_sb[j][:, lcol],
                                        start=(j == 0),
                                        stop=(j == NJ - 1),
                                    )
                                rope(ps, q_sb[h][:, cols], cols)
                            # K^T
                            ps = ps1.tile([128, SC], f32, tag="ps", name="ps")
                            for j in range(NJ):
                                nc.tensor.matmul(
                                    ps[:], wk_sb[j][:], xt_sb[j][:, lcol],
                                    start=(j == 0), stop=(j == NJ - 1),
                                )
                            rope(ps, k_sb[:, cols], cols)
                            # V^T then PE-transpose into V
                            ps = ps1.tile([128, SC], f32, tag="ps", name="ps")
                            for j in range(NJ):
                                nc.tensor.matmul(
                                    ps[:], wv_sb[j][:], xt_sb[j][:, lcol],
                                    start=(j == 0), stop=(j == NJ - 1),
                                )
                            vt = ropep.tile([D, SC], bf16, tag="vt", name="vt")
                            nc.scalar.copy(vt[:], ps[:])
                            for t in range(SC // 128):
                                st = sc * (SC // 128) + t
                                trp = pstr.tile([128, 128], bf16, tag="tr", name="tr")
                                nc.tensor.transpose(
                                    trp[:], vt[:, t * 128 : (t + 1) * 128], ident_sb[:]
                                )
                                nc.scalar.copy(v_sb[:, st * 128 : (st + 1) * 128], trp[:])

                # prefetch Wo column-slice (overlaps attention)
                wo_sb = [wop.tile([128, FC], bf16, tag="wo", name=f"wo{_}") for _ in range(NJ)]
                for j in range(NJ):
                    nc.sync.dma_start(out=wo_sb[j][:], in_=woT[j * 128 : (j + 1) * 128, :])

                # ---------------- phase 2: causal attention ----------------
                with (
                    tc.tile_pool(name="s", bufs=3, space="PSUM") as sp,
                    tc.tile_pool(name="att", bufs=2, space="PSUM") as attp,
                    tc.tile_pool(name="l", bufs=2, space="PSUM") as lp,
                    tc.tile_pool(name="p", bufs=6) as pp,
                    tc.tile_pool(name="ao", bufs=3) as aop,
                    tc.tile_pool(name="rc", bufs=3) as rcp,
                    tc.tile_pool(name="bc", bufs=1, space="PSUM") as bcp,
                    tc.tile_pool(name="bcs", bufs=2) as bcsp,
                ):
                    for qc in range(NSC):
                        qcols = slice(qc * SC, (qc + 1) * SC)
                        nkt = 4 * (qc + 1)
                        for g in range(HPC // 2):
                            pair = (2 * g, 2 * g + 1)
                            att_ps = {h: attp.tile([D, SC], f32, tag="att", name=f"att{h}") for h in pair}
                            l_ps = {h: lp.tile([1, SC], f32, tag="l", name=f"l{h}") for h in pair}
                            for kt in range(nkt):
                                kcols = slice(kt * 128, (kt + 1) * 128)
                                s_ps, p_sb = {}, {}
                                for h in pair:
                                    s_ps[h] = sp.tile([128, SC], f32, tag="s", name=f"s{h}")
                                    nc.tensor.matmul(
                                        s_ps[h][:], k_sb[:, kcols], q_sb[h][:, qcols],
                                        start=True, stop=True,
                                    )
                                for h in pair:
                                    p_sb[h] = pp.tile([128, SC], bf16, tag="p", name=f"p{h}")
                                    nc.scalar.activation(
                                        p_sb[h][:], s_ps[h][:], Exp,
                                        bias=bias_sb[:], scale=float(SCALE),
                                    )
                                    jd = kt - 4 * qc
                                    if jd >= 0:
                                        nc.vector.tensor_mul(
                                            p_sb[h][:], p_sb[h][:],
                                            dm_sb[:, jd * SC : (jd + 1) * SC],
                                        )
                                first, last = kt == 0, kt == nkt - 1
                                for h in pair:
                                    nc.tensor.matmul(
                                        att_ps[h][:], v_sb[:, kcols], p_sb[h][:],
                                        start=first, stop=last,
                                    )
                                    nc.tensor.matmul(
                                        l_ps[h][:], ones_sb[:, 0:1], p_sb[h][:],
                                        start=first, stop=last,
                                    )
                            for h in pair:
                                rc = rcp.tile([1, SC], f32, tag="rc", name="rc")
                                nc.vector.reciprocal(rc[:], l_ps[h][:])
                                bc = bcp.tile([D, SC], f32, tag="bc", name="bc")
                                nc.tensor.matmul(bc[:], ones_row[:], rc[:], start=True, stop=True)
                                bcs = bcsp.tile([D, SC], bf16, tag="bcs", name="bcs")
                                nc.scalar.copy(bcs[:], bc[:])
                                ao = aop.tile([D, SC], bf16, tag="ao", name="ao")
                                nc.vector.tensor_mul(ao[:], att_ps[h][:], bcs[:])
                                nc.sync.dma_start(
                                    out=ag_in[h * 128 : (h + 1) * 128, qcols], in_=ao[:]
                                )

            import concourse.mybir as _mb

            nc.gpsimd.collective_compute(
                "AllGather",
                _mb.AluOpType.bypass,
                replica_groups=[list(range(NC))],
                ins=[ag_in.ap()],
                outs=[ag_out.ap()],
            )

            # ---------------- phase 3: output projection (column slice) ----------------
            with (
                tc.tile_pool(name="ag", bufs=NJ) as agp,
                tc.tile_pool(name="ps3", bufs=4, space="PSUM") as ps3,
                tc.tile_pool(name="os", bufs=3) as osp,
            ):
                ag_sb = [agp.tile([128, S], bf16, tag="ag", name=f"ag{_}") for _ in range(NJ)]
                for j in range(NJ):
                    nc.sync.dma_start(
                        out=ag_sb[j][:], in_=ag_out[j * 128 : (j + 1) * 128, :]
                    )
                for f in range(HPC):
                    frows = slice(f * 128, (f + 1) * 128)
                    for sc in range(NSC):
                        cols = slice(sc * SC, (sc + 1) * SC)
                        ps = ps3.tile([128, SC], f32, tag="ps3", name="ps3")
                        for j in range(NJ):
                            nc.tensor.matmul(
                                ps[:], wo_sb[j][:, frows], ag_sb[j][:, cols],
                                start=(j == 0), stop=(j == NJ - 1),
                            )
                        ot = osp.tile([128, SC], f32, tag="os", name="os")
                        nc.scalar.copy(ot[:], ps[:])
                        nc.sync.dma_start(out=outT[frows, cols], in_=ot[:])

    nc.compile()
    return nc


def _get_compiled():
    global _compiled
    if _compiled is None:
        _compiled = _build_nc()
    return _compiled


def _rope_tables(position_ids):
    pos = np.asarray(position_ids).reshape(-1).astype(np.float32)
    inv_freq = (1.0 / (THETA ** (np.arange(0, D, 2, dtype=np.float32) / D))).astype(
        np.float32
    )
    freqs = np.outer(pos, inv_freq)
    emb = np.concatenate([freqs, freqs], axis=-1)  # [S, D]
    return np.cos(emb).astype(np.float32), np.sin(emb).astype(np.float32)


def _is_causal(mask):
    m = np.asarray(mask)[0, 0]
    if m.shape != (S, S):
        return False
    tri = np.tril(np.ones((S, S), dtype=bool))
    return bool((m[tri] == 0.0).all() and (m[~tri] < -1e30).all())


def _numpy_reference(hidden_states, attention_mask, position_ids, Wq, Wk, Wv, Wo):
    x = np.asarray(hidden_states, np.float32)
    b, s, hid = x.shape
    n_rep = H // KVH
    q = (x @ Wq.T).reshape(b, s, H, D).transpose(0, 2, 1, 3)
    k = (x @ Wk.T).reshape(b, s, KVH, D).transpose(0, 2, 1, 3)
    v = (x @ Wv.T).reshape(b, s, KVH, D).transpose(0, 2, 1, 3)
    cos_t, sin_t = _rope_tables(position_ids)
    cos = cos_t[None, None]
    sin = sin_t[None, None]

    def rot(t):
        return np.concatenate([-t[..., D // 2 :], t[..., : D // 2]], axis=-1)

    q = q * cos + rot(q) * sin
    k = k * cos + rot(k) * sin
    k = np.repeat(k, n_rep, axis=1)
    v = np.repeat(v, n_rep, axis=1)
    scores = np.einsum("bhqd,bhkd->bhqk", q, k) / np.sqrt(np.float32(D))
    scores = scores + np.asarray(attention_mask, np.float32)
    scores = scores - scores.max(axis=-1, keepdims=True)
    p = np.exp(scores)
    p = p / p.sum(axis=-1, keepdims=True)
    attn = np.einsum("bhqk,bhkd->bhqd", p, v)
    attn = attn.transpose(0, 2, 1, 3).reshape(b, s, H * D)
    return (attn @ Wo.T).astype(np.float32)


def kernel(hidden_states, attention_mask, position_ids, Wq, Wk, Wv, Wo):
    hidden_states = np.asarray(hidden_states)
    if not _is_causal(attention_mask):
        return _numpy_reference(
            hidden_states, attention_mask, position_ids, Wq, Wk, Wv, Wo
        )

    from concourse import bass_utils

    nc = _get_compiled()

    xTb = np.ascontiguousarray(
        np.asarray(hidden_states, np.float32)[0].T
    ).astype(_BF16)
    cos_t, sin_t = _rope_tables(position_ids)
    cosT = np.ascontiguousarray(cos_t.T).astype(_BF16)
    sinT_s = np.ascontiguousarray(sin_t.T)
    sinT_s[: D // 2] *= -1.0
    sinT_s = sinT_s.astype(_BF16)
    dm = np.zeros((D, 4 * SC), np.float32)
    ki = np.arange(D)[:, None]
    qi = np.arange(SC)[None, :]
    for j in range(4):
        dm[:, j * SC : (j + 1) * SC] = (ki <= qi - 128 * j).astype(np.float32)
    dm = dm.astype(_BF16)

    Wq32 = np.asarray(Wq, np.float32)
    Wk32 = np.asarray(Wk, np.float32)
    Wv32 = np.asarray(Wv, np.float32)
    Wo32 = np.asarray(Wo, np.float32)

    in_maps = []
    for c in range(NC):
        in_maps.append(
            dict(
                xT=xTb,
                wqT=np.ascontiguousarray(Wq32[c * FC : (c + 1) * FC, :].T).astype(_BF16),
                wkT=np.ascontiguousarray(Wk32[c * D : (c + 1) * D, :].T).astype(_BF16),
                wvT=np.ascontiguousarray(Wv32[c * D : (c + 1) * D, :].T).astype(_BF16),
                woT=np.ascontiguousarray(Wo32.T[:, c * FC : (c + 1) * FC]).astype(_BF16),
                cosT=cosT,
                sinT=sinT_s,
                dmask=dm,
            )
        )

    res = bass_utils.run_bass_kernel_spmd(nc, in_maps, core_ids=list(range(NC)))
    oT = np.concatenate([res.results[c]["outT"] for c in range(NC)], axis=0)
    return np.ascontiguousarray(oT.T)[None].astype(np.float32)
